# revision 7
# baseline (speedup 1.0000x reference)
"""Causal multi-head attention on 8 Trainium2 NeuronCores.

Sharding: 8 cores = 4 batches x 2 head-halves. Each core handles one batch
and 8 of the 16 heads (feature range hf*512 .. hf*512+512 of the QKV
projections), computes a partial output projection [2048, 1024], and the
host sums the two half-partials per batch and adds the bias.

Per-core kernel (all matmuls in float32r = TF32-like, 4x fp32 PE rate):
  - qT/kT per head-pair [128, 2048] via W.T @ x.T   (transposed layout)
  - v natural [tok, 8, 64+ones] for the PV matmul
  - scoresT[keys, q] = kT.T @ qT per 128-key block, 2 heads packed in the
    PE array via row tiling (K=64 each at tile_position (0,0)/(64,0))
  - causal mask: QK N narrowed below the diagonal; -1e30 triangular mask
    added on the 128-wide diagonal blocks in PSUM before exp
  - exp on ACT (scale=1/8 folded); PV with lhsT=[v|1] (M=65) accumulates
    ctxT and the softmax denominator in one PSUM tile
  - denominator broadcast across partitions with a K=1 ones matmul, then
    one DVE divide writes normalized ctxT
  - output projection ctxT.T @ Wp-slice -> partial [2048, 1024]
"""

import numpy as np

B, S, D = 4, 2048, 1024

_CACHE = {}


def _build(R=1):
    import concourse.bacc as bacc
    import concourse.tile as tile
    import concourse.mybir as mybir
    from concourse.bass import ts, ds

    fr = mybir.dt.float32r
    f32 = mybir.dt.float32
    Exp = mybir.ActivationFunctionType.Exp
    Alu = mybir.AluOpType

    nc = bacc.Bacc("TRN2", target_bir_lowering=False)
    xT_d = nc.dram_tensor("xT", [128, 8, S], fr, kind="ExternalInput")
    wq_d = nc.dram_tensor("wq", [128, 8, 4, 128], fr, kind="ExternalInput")
    wk_d = nc.dram_tensor("wk", [128, 8, 4, 128], fr, kind="ExternalInput")
    wv_d = nc.dram_tensor("wv", [128, 8, 512], fr, kind="ExternalInput")
    wp_d = nc.dram_tensor("wp", [128, 4, 1024], fr, kind="ExternalInput")
    mk_d = nc.dram_tensor("mask", [128, 128], f32, kind="ExternalInput")
    out_d = nc.dram_tensor("out", [16, 128, 1024], f32, kind="ExternalOutput")

    with tile.TileContext(nc) as tc:
        with (
            tc.tile_pool(name="const", bufs=1) as cpool,
            tc.tile_pool(name="wqk", bufs=4) as wqkp,
            tc.tile_pool(name="wvp", bufs=1) as wvpp,
            tc.tile_pool(name="chunks", bufs=2) as chp,
            tc.tile_pool(name="qk", bufs=2) as qkp,
            tc.tile_pool(name="xp", bufs=3) as xpp,
            tc.tile_pool(name="sums", bufs=2) as smp,
            tc.tile_pool(name="ost", bufs=2) as ostp,
            tc.tile_pool(name="sc", bufs=2, space="PSUM") as scp,
            tc.tile_pool(name="ctx", bufs=2, space="PSUM") as ctxp,
            tc.tile_pool(name="pj", bufs=2, space="PSUM") as pjp,
        ):
            mask_sb = cpool.tile([128, 128], f32, tag="mask")
            nc.sync.dma_start(mask_sb[:], mk_d[:])
            ones32 = cpool.tile([128, 128], f32, tag="ones32")
            nc.vector.memset(ones32[:], 1.0)
            ones_row = cpool.tile([1, 64], fr, tag="ones")
            nc.vector.tensor_copy(ones_row[:], ones32[0:1, 0:64])
            v_sb = cpool.tile([128, 16, 8, 65], fr, tag="v")
            nc.vector.tensor_copy(
                v_sb[:, :, :, ds(64, 1)], ones32[:, 0:128]
            )
            ctxT_sb = cpool.tile([128, 4, S], fr, tag="ctxT")

            def v_proj(wv_sb):
                for cb in range(4):
                    ch = chp.tile([128, 8, 512], fr, tag="ch")
                    nc.sync.dma_start(ch[:], xT_d[:, :, ts(cb, 512)])
                    for sb4 in range(4):
                        tb = cb * 4 + sb4
                        pv = pjp.tile([128, 8, 64], f32, tag="pj")
                        for kc in range(8):
                            nc.tensor.matmul(
                                pv[:],
                                ch[:, kc, ts(sb4, 128)],
                                wv_sb[:, kc, :],
                                start=(kc == 0),
                                stop=(kc == 7),
                            )
                        nc.vector.tensor_copy(v_sb[:, tb, :, 0:64], pv[:])

            def qk_proj(hp):
                wqt = wqkp.tile([128, 8, 128], fr, tag="wqk")
                nc.sync.dma_start(wqt[:], wq_d[:, :, hp, :])
                wkt = wqkp.tile([128, 8, 128], fr, tag="wqk")
                nc.sync.dma_start(wkt[:], wk_d[:, :, hp, :])
                qT = qkp.tile([128, S], fr, tag="qT")
                kT = qkp.tile([128, S], fr, tag="kT")
                for cb in range(4):
                    ch = chp.tile([128, 8, 512], fr, tag="ch")
                    nc.sync.dma_start(ch[:], xT_d[:, :, ts(cb, 512)])
                    for wt, dst in ((wqt, qT), (wkt, kT)):
                        pq = pjp.tile([128, 512], f32, tag="pj")
                        for kc in range(8):
                            nc.tensor.matmul(
                                pq[:],
                                wt[:, kc, :],
                                ch[:, kc, :],
                                start=(kc == 0),
                                stop=(kc == 7),
                            )
                        nc.vector.tensor_copy(dst[:, ts(cb, 512)], pq[:])
                return qT, kT

            def attn(hp, qT, kT):
                for qt in range(4):
                    ctx = {
                        h: ctxp.tile([128, 512], f32, tag="ctx", name=f"ctx{h}")
                        for h in (0, 1)
                    }
                    n_kb = 4 * qt + 4
                    for g in range(n_kb // 2):
                        sc = {
                            h: scp.tile([128, 2, 512], f32, tag="sc", name=f"sc{h}")
                            for h in (0, 1)
                        }
                        qoffs = []
                        for i in (0, 1):
                            j = 2 * g + i
                            m = j - 4 * qt
                            qoff = 128 * m if m > 0 else 0
                            qoffs.append(qoff)
                            for h in (0, 1):
                                nc.tensor.matmul(
                                    sc[h][:, i, qoff:512],
                                    kT[ds(64 * h, 64), ts(j, 128)],
                                    qT[ds(64 * h, 64), ds(qt * 512 + qoff, 512 - qoff)],
                                    start=True,
                                    stop=True,
                                    tile_position=(64 * h, 0),
                                )
                        for i in (0, 1):
                            j = 2 * g + i
                            m = j - 4 * qt
                            if m >= 0:
                                for h in (0, 1):
                                    nc.vector.tensor_tensor(
                                        out=sc[h][:, i, ds(128 * m, 128)],
                                        in0=sc[h][:, i, ds(128 * m, 128)],
                                        in1=mask_sb[:],
                                        op=Alu.add,
                                    )
                        xps = {}
                        for h in (0, 1):
                            xp = xpp.tile([128, 2, 512], fr, tag="xp")
                            if qoffs[0] == 0 and qoffs[1] == 0:
                                nc.scalar.activation(xp[:, :, :], sc[h][:, :, :], Exp, scale=0.125)
                            else:
                                for i in (0, 1):
                                    qo = qoffs[i]
                                    nc.scalar.activation(
                                        xp[:, i, qo:512], sc[h][:, i, qo:512], Exp, scale=0.125
                                    )
                            xps[h] = xp
                        for i in (0, 1):
                            j = 2 * g + i
                            qo = qoffs[i]
                            for h in (0, 1):
                                nc.tensor.matmul(
                                    ctx[h][0:65, qo:512],
                                    v_sb[:, j, 2 * hp + h, :],
                                    xps[h][:, i, qo:512],
                                    start=(j == 0),
                                    stop=(j == n_kb - 1),
                                    skip_group_check=True,
                                )
                    for h in (0, 1):
                        rc = smp.tile([1, 512], f32, tag="rc")
                        nc.vector.reciprocal(rc[:], ctx[h][ds(64, 1), :])
                        sm = smp.tile([1, 512], fr, tag="sm")
                        nc.vector.tensor_copy(sm[:], rc[:])
                        rb = pjp.tile([128, 512], f32, tag="pj")
                        nc.tensor.matmul(rb[0:64, :], ones_row[:], sm[:], start=True, stop=True)
                        rbs = smp.tile([64, 512], f32, tag="rbs")
                        nc.vector.tensor_copy(rbs[:], rb[0:64, :])
                        nc.vector.tensor_tensor(
                            out=ctxT_sb[ds(64 * h, 64), hp, ts(qt, 512)],
                            in0=ctx[h][0:64, :],
                            in1=rbs[:],
                            op=Alu.mult,
                        )

            def out_proj(wp_sb):
                for tt in range(16):
                    for ncv in range(2):
                        po = pjp.tile([128, 512], f32, tag="pj")
                        for fc in range(4):
                            nc.tensor.matmul(
                                po[:],
                                ctxT_sb[:, fc, ts(tt, 128)],
                                wp_sb[:, fc, ds(ncv * 512, 512)],
                                start=(fc == 0),
                                stop=(fc == 3),
                            )
                        ot = ostp.tile([128, 512], f32, tag="ost")
                        nc.vector.tensor_copy(ot[:], po[:])
                        nc.sync.dma_start(out_d[tt, :, ds(ncv * 512, 512)], ot[:])

            def body():
                wv_sb = wvpp.tile([128, 8, 512], fr, tag="wvp")
                nc.sync.dma_start(wv_sb[:], wv_d[:])
                v_proj(wv_sb)
                for hp in range(4):
                    qT, kT = qk_proj(hp)
                    attn(hp, qT, kT)
                wp_sb = wvpp.tile([128, 4, 1024], fr, tag="wvp")
                nc.sync.dma_start(wp_sb[:], wp_d[:])
                out_proj(wp_sb)

            if R > 1:
                with tc.For_i(0, R):
                    body()
            else:
                body()

    nc.compile()
    return nc


def _get_program(R=1):
    if R not in _CACHE:
        _CACHE[R] = _build(R)
    return _CACHE[R]


def _shard_inputs(x, Wq, Wk, Wv, Wp):
    x = np.ascontiguousarray(x, dtype=np.float32)
    mask = np.where(
        np.arange(128)[:, None] > np.arange(128)[None, :], -1.0e30, 0.0
    ).astype(np.float32)
    in_maps = []
    for c in range(8):
        b, hf = c // 2, c % 2
        hs = slice(hf * 512, hf * 512 + 512)
        xT = np.ascontiguousarray(
            x[b].T.reshape(8, 128, S).transpose(1, 0, 2)
        )
        wq = np.ascontiguousarray(Wq[hs].T.reshape(8, 128, 4, 128).transpose(1, 0, 2, 3))
        wk = np.ascontiguousarray(Wk[hs].T.reshape(8, 128, 4, 128).transpose(1, 0, 2, 3))
        wv = np.ascontiguousarray(Wv[hs].T.reshape(8, 128, 512).transpose(1, 0, 2))
        wp = np.ascontiguousarray(Wp.T[hs].reshape(4, 128, D).transpose(1, 0, 2))
        in_maps.append(
            {"xT": xT, "wq": wq, "wk": wk, "wv": wv, "wp": wp, "mask": mask}
        )
    return in_maps


def kernel(x, Wq, Wk, Wv, Wp, bp, _R=1, _return_res=False):
    from concourse.bass_utils import run_bass_kernel_spmd

    nc = _get_program(_R)
    in_maps = _shard_inputs(x, Wq, Wk, Wv, Wp)
    res = run_bass_kernel_spmd(nc, in_maps, list(range(8)))
    out = np.empty((B, S, D), dtype=np.float32)
    for b in range(B):
        p0 = res.results[2 * b]["out"].reshape(S, D)
        p1 = res.results[2 * b + 1]["out"].reshape(S, D)
        out[b] = p0 + p1 + bp.astype(np.float32)
    if _return_res:
        return out, res
    return out


# revision 55
# speedup vs baseline: 238.1135x; 238.1135x over previous
"""Causal multi-head attention on 8 Trainium2 NeuronCores.

Sharding: 8 cores = 4 batches x 2 head-halves. Each core handles one batch
and 8 of the 16 heads (feature range hf*512 .. hf*512+512 of the QKV
projections), computes a partial output projection [2048, 1024], and the
host sums the two half-partials per batch and adds the bias.

Per-core kernel:
  - single sweep over x.T chunks computes qT/kT (bf16, [128, 2048] per
    head-pair) and v (bf16, [tok, head, 64|1] with a ones column)
  - attention qt-outer / head-pair-inner; scoresT[keys, q] = kT.T @ qT per
    128-key block (bf16, fp32 PSUM); causal mask via narrowed QK plus a
    -1e30 triangular add on diagonal blocks; exp on ACT (scale=1/8 folded),
    output bf16; PV with lhsT=[v|1] (M=65) accumulates ctxT and the softmax
    denominator in one PSUM tile; PV deferred 4 tiles behind exp to keep
    engine streams from blocking each other
  - per-qt: unnormalized ctxT copied to SBUF (fp32r), denominator
    reciprocal broadcast via a K=1 ones matmul, normalization multiply on
    GpSimd; output projection for that qt's tokens (fp32r) emitted one qt
    behind, so it fills PE gaps under the ACT-bound attention
"""

import numpy as np

B, S, D = 4, 2048, 1024

_CACHE = {}


def _build(R=1, mode="full"):
    import concourse.bacc as bacc
    import concourse.tile as tile
    import concourse.mybir as mybir
    from concourse.bass import ts, ds

    fr = mybir.dt.float32r
    f32 = mybir.dt.float32
    bf = mybir.dt.bfloat16
    Exp = mybir.ActivationFunctionType.Exp
    Alu = mybir.AluOpType

    nc = bacc.Bacc("TRN2", target_bir_lowering=False)
    xT_d = nc.dram_tensor("xT", [128, 8, S], bf, kind="ExternalInput")
    wq_d = nc.dram_tensor("wq", [128, 8, 4, 128], bf, kind="ExternalInput")
    wk_d = nc.dram_tensor("wk", [128, 8, 4, 128], bf, kind="ExternalInput")
    wv_d = nc.dram_tensor("wv", [128, 8, 512], bf, kind="ExternalInput")
    wp_d = nc.dram_tensor("wp", [128, 4, 1024], fr, kind="ExternalInput")
    mk_d = nc.dram_tensor("mask", [128, 128], f32, kind="ExternalInput")
    mk01_d = nc.dram_tensor("mask01", [128, 128], bf, kind="ExternalInput")
    out_d = nc.dram_tensor("out", [16, 128, 1024], f32, kind="ExternalOutput")

    with tile.TileContext(nc) as tc:
        with (
            tc.tile_pool(name="const", bufs=1) as cpool,
            tc.tile_pool(name="chunks", bufs=2) as chp,
            tc.tile_pool(name="xp", bufs=8) as xpp,
            tc.tile_pool(name="sums", bufs=9) as smp,
            tc.tile_pool(name="rrp", bufs=2) as rrp,
            tc.tile_pool(name="ctxt", bufs=3) as ctxtp,
            tc.tile_pool(name="ost", bufs=2) as ostp,
            tc.tile_pool(name="sc", bufs=2, space="PSUM") as scp,
            tc.tile_pool(name="ctx", bufs=2, space="PSUM") as ctxp,
            tc.tile_pool(name="pj", bufs=2, space="PSUM") as pjp,
        ):
            mask_sb = cpool.tile([128, 128], f32, tag="mask")
            nc.sync.dma_start(mask_sb[:], mk_d[:])
            m01_sb = cpool.tile([128, 128], bf, tag="m01")
            nc.sync.dma_start(m01_sb[:], mk01_d[:])
            ones32 = cpool.tile([128, 128], f32, tag="ones32")
            nc.vector.memset(ones32[:], 1.0)
            ones_row = cpool.tile([1, 64], fr, tag="ones")
            nc.vector.tensor_copy(ones_row[:], ones32[0:1, 0:64])
            v_sb = cpool.tile([128, 16, 8, 65], bf, tag="v")
            nc.vector.tensor_copy(v_sb[:, :, :, ds(64, 1)], ones32[:, 0:128])
            ctxp_sb = None
            # persistent qT/kT for all 4 head-pairs
            qTs, kTs, qTds, kTds = [], [], [], []
            for hp in range(4):
                qTs.append(cpool.tile([128, S], bf, tag=f"qT{hp}", name=f"qT{hp}"))
                kTs.append(cpool.tile([128, S], bf, tag=f"kT{hp}", name=f"kT{hp}"))
                qTds.append(cpool.tile([128, S], bf, tag=f"qTd{hp}", name=f"qTd{hp}"))
                kTds.append(cpool.tile([128, S], bf, tag=f"kTd{hp}", name=f"kTd{hp}"))
            wv_sb = cpool.tile([128, 8, 512], bf, tag="wv")
            wp_sb = cpool.tile([128, 4, 1024], fr, tag="wp")
            wqt_sb, wkt_sb = [], []
            for hp in range(4):
                wqt_sb.append(
                    cpool.tile([128, 8, 128], bf, tag=f"wq{hp}", name=f"wq{hp}")
                )
                wkt_sb.append(
                    cpool.tile([128, 8, 128], bf, tag=f"wk{hp}", name=f"wk{hp}")
                )

            def proj_setup():
                # hp0 weights first on the SP queue (gate the first proj
                # groups); the rest streams in parallel on the ACT HWDGE queue
                nc.sync.dma_start(wqt_sb[0][:], wq_d[:, :, 0, :])
                nc.sync.dma_start(wkt_sb[0][:], wk_d[:, :, 0, :])
                nc.scalar.dma_start(wv_sb[:], wv_d[:])
                for hp in range(1, 4):
                    nc.scalar.dma_start(wqt_sb[hp][:], wq_d[:, :, hp, :])
                    nc.scalar.dma_start(wkt_sb[hp][:], wk_d[:, :, hp, :])

            def proj_cb_thunks(cb):
                box = {}

                def dma_chunk():
                    ch = chp.tile([128, 8, 512], bf, tag="ch")
                    nc.sync.dma_start(ch[:], xT_d[:, :, ts(cb, 512)])
                    box["ch"] = ch

                thunks = [dma_chunk]

                def qk_group(wt, dst, dstd):
                    ch = box["ch"]
                    pq = pjp.tile([128, 512], f32, tag="pj")
                    for kc in range(8):
                        nc.tensor.matmul(
                            pq[:],
                            wt[:, kc, :],
                            ch[:, kc, :],
                            start=(kc == 0),
                            stop=(kc == 7),
                        )
                    nc.vector.tensor_copy(dst[:, ts(cb, 512)], pq[:])
                    nc.sync.dma_start(
                        dstd[ds(64, 64), ts(cb, 512)], dst[ds(0, 64), ts(cb, 512)]
                    )
                    nc.sync.dma_start(
                        dstd[ds(0, 64), ts(cb, 512)], dst[ds(64, 64), ts(cb, 512)]
                    )

                def v_group(sb4):
                    ch = box["ch"]
                    tb = cb * 4 + sb4
                    pv = pjp.tile([128, 8, 64], f32, tag="pj")
                    for kc in range(8):
                        nc.tensor.matmul(
                            pv[:],
                            ch[:, kc, ts(sb4, 128)],
                            wv_sb[:, kc, :],
                            start=(kc == 0),
                            stop=(kc == 7),
                        )
                    nc.vector.tensor_copy(v_sb[:, tb, :, 0:64], pv[:])

                import functools

                for hp in range(4):
                    for wt, dst, dstd in (
                        (wqt_sb[hp], qTs[hp], qTds[hp]),
                        (wkt_sb[hp], kTs[hp], kTds[hp]),
                    ):
                        thunks.append(functools.partial(qk_group, wt, dst, dstd))
                for sb4 in range(4):
                    thunks.append(functools.partial(v_group, sb4))
                return thunks

            def proj_cb(cb):
                for t in proj_cb_thunks(cb):
                    t()

            def attn_qt(hp, qt, sums, fill=None, ctile=None):
                """Attention for one (head-pair, query-tile of 512)."""
                qT, kT = qTs[hp], kTs[hp]
                ctx = {
                    h: ctxp.tile([128, 512], f32, tag="ctx", name=f"ctx{h}")
                    for h in (0, 1)
                }
                n_kb = 4 * qt + 4
                pend = []

                def emit_pv(item):
                    h, xp_, qoffs_, last, j0 = item
                    for i in (0, 1):
                        qo = qoffs_[i]
                        nc.tensor.matmul(
                            ctx[h][0:65, qo:512],
                            v_sb[:, j0 + i, 2 * hp + h, :],
                            xp_[:, i, qo:512],
                            start=(j0 == 0 and i == 0),
                            stop=(last and i == 1),
                            skip_group_check=True,
                        )

                prev_xp = None
                for h in (0, 1):
                    for g in range(n_kb // 2):
                        qoffs = []
                        for i in (0, 1):
                            m = 2 * g + i - 4 * qt
                            qoffs.append(128 * m if m > 0 else 0)
                        sc1 = scp.tile([128, 2, 512], f32, tag="sc", name=f"sc{h}")
                        for i in (0, 1):
                            j = 2 * g + i
                            qoff = qoffs[i]
                            if i == 0 or mode == "nopack":
                                kk, qq, base = kT, qT, 64 * h
                            else:
                                kk, qq, base = kTds[hp], qTds[hp], 64 * (1 - h)
                            nc.tensor.matmul(
                                sc1[:, i, qoff:512],
                                kk[ds(base, 64), ts(j, 128)],
                                qq[ds(base, 64), ds(qt * 512 + qoff, 512 - qoff)],
                                start=True,
                                stop=True,
                                tile_position=(base, 0),
                            )
                        if h == 1 and mode == "halfexp" and prev_xp is not None:
                            xp = prev_xp
                        else:
                            xp = xpp.tile([128, 2, 512], bf, tag="xp")
                            nc.scalar.activation(
                                xp[:, :, :], sc1[:, :, :], Exp, scale=0.125
                            )
                            for i in (0, 1):
                                m = 2 * g + i - 4 * qt
                                if m >= 0:
                                    nc.vector.tensor_tensor(
                                        out=xp[:, i, ds(128 * m, 128)],
                                        in0=xp[:, i, ds(128 * m, 128)],
                                        in1=m01_sb[:],
                                        op=Alu.mult,
                                    )
                        prev_xp = xp
                        pend.append((h, xp, qoffs, g == n_kb // 2 - 1, 2 * g))
                        while len(pend) > 6:
                            emit_pv(pend.pop(0))
                        if fill:
                            fill.popleft()()
                while pend:
                    emit_pv(pend.pop(0))
                # unnormalized ctxT + sums out
                for h in (0, 1):
                    sm = smp.tile([1, 512], fr, tag="sm", name=f"sm{h}")
                    nc.vector.tensor_copy(sm[:], ctx[h][ds(64, 1), :])
                    sums[(hp, h)] = sm
                    nc.vector.tensor_copy(
                        ctile[ds(64 * h, 64), hp, :], ctx[h][0:64, :]
                    )

            def normalize_qt(qt, sums, ctile):
                for hp in range(4):
                    for h in (0, 1):
                        rb = pjp.tile([128, 512], f32, tag="pj", name=f"rb{h}")
                        nc.tensor.matmul(
                            rb[0:64, :],
                            ones_row[:],
                            sums[(hp, h)][:],
                            start=True,
                            stop=True,
                        )
                        rr = rrp.tile([128, 512], fr, tag="rr", name=f"rr{h}")
                        with nc.allow_low_precision(reason="fp32r recip for mult"):
                            nc.vector.reciprocal(rr[ds(64 * h, 64), :], rb[0:64, :])
                        dst = ctile[ds(64 * h, 64), hp, :]
                        nc.gpsimd.tensor_tensor(
                            out=dst, in0=dst, in1=rr[ds(64 * h, 64), :], op=Alu.mult
                        )

            def out_proj_group(ctile, tt, ncv):
                po = pjp.tile([128, 512], f32, tag="pj")
                for fc in range(4):
                    nc.tensor.matmul(
                        po[:],
                        ctile[:, fc, ts(tt % 4, 128)],
                        wp_sb[:, fc, ds(ncv * 512, 512)],
                        start=(fc == 0),
                        stop=(fc == 3),
                    )
                ot = ostp.tile([128, 512], f32, tag="ost")
                nc.vector.tensor_copy(ot[:], po[:])
                nc.sync.dma_start(out_d[tt, :, ds(ncv * 512, 512)], ot[:])

            def out_proj_thunks(qt, ctile):
                import functools

                return [
                    functools.partial(out_proj_group, ctile, qt * 4 + tt4, ncv)
                    for tt4 in range(4)
                    for ncv in range(2)
                ]

            def out_proj_qt(qt, ctile):
                for t in out_proj_thunks(qt, ctile):
                    t()

            def body():
                proj_setup()
                if mode == "noattn":
                    for cb in range(4):
                        proj_cb(cb)
                    return
                from collections import deque

                nc.scalar.dma_start(wp_sb[:], wp_d[:])
                proj_cb(0)
                fill = deque()
                prev = None
                prev_ct = None
                for qt in range(4):
                    ctile = ctxtp.tile([128, 4, 512], fr, tag="ctxt", name=f"ct{qt}")
                    if qt < 3:
                        fill.extend(proj_cb_thunks(qt + 1))
                    if prev is not None:
                        fill.extend(out_proj_thunks(prev, prev_ct))
                    sums = {}
                    for hp in range(4):
                        attn_qt(hp, qt, sums, fill, ctile)
                    normalize_qt(qt, sums, ctile)
                    prev = qt
                    prev_ct = ctile
                while fill:
                    fill.popleft()()
                out_proj_qt(prev, prev_ct)

            if mode == "attnloop":
                proj_setup()
                for cb in range(4):
                    proj_cb(cb)
                nc.sync.dma_start(wp_sb[:], wp_d[:])

                def attn_body():
                    prev = None
                    prev_ct = None
                    for qt in range(4):
                        ctile = ctxtp.tile([128, 4, 512], fr, tag="ctxt", name=f"ct{qt}")
                        sums = {}
                        for hp in range(4):
                            attn_qt(hp, qt, sums, ctile=ctile)
                        normalize_qt(qt, sums, ctile)
                        if prev is not None:
                            out_proj_qt(prev, prev_ct)
                        prev = qt
                        prev_ct = ctile
                    out_proj_qt(prev, prev_ct)

                if R > 1:
                    with tc.For_i(0, R):
                        attn_body()
                else:
                    attn_body()
            elif R > 1:
                with tc.For_i(0, R):
                    body()
            else:
                body()

    nc.compile()
    return nc


def _get_program(R=1, mode="full"):
    key = (R, mode)
    if key not in _CACHE:
        _CACHE[key] = _build(R, mode)
    return _CACHE[key]


def _shard_inputs(x, Wq, Wk, Wv, Wp):
    import ml_dtypes

    bf = ml_dtypes.bfloat16
    x = np.ascontiguousarray(x, dtype=np.float32)
    mask = np.where(
        np.arange(128)[:, None] > np.arange(128)[None, :], -1.0e30, 0.0
    ).astype(np.float32)
    mask01 = np.where(
        np.arange(128)[:, None] > np.arange(128)[None, :], 0.0, 1.0
    ).astype(bf)
    in_maps = []
    for c in range(8):
        b, hf = c // 2, c % 2
        hs = slice(hf * 512, hf * 512 + 512)
        xT = np.ascontiguousarray(
            x[b].T.reshape(8, 128, S).transpose(1, 0, 2)
        ).astype(bf)
        wq = np.ascontiguousarray(
            Wq[hs].T.reshape(8, 128, 4, 128).transpose(1, 0, 2, 3)
        ).astype(bf)
        wk = np.ascontiguousarray(
            Wk[hs].T.reshape(8, 128, 4, 128).transpose(1, 0, 2, 3)
        ).astype(bf)
        wv = np.ascontiguousarray(
            Wv[hs].T.reshape(8, 128, 512).transpose(1, 0, 2)
        ).astype(bf)
        wp = np.ascontiguousarray(Wp.T[hs].reshape(4, 128, D).transpose(1, 0, 2))
        in_maps.append(
            {
                "xT": xT, "wq": wq, "wk": wk, "wv": wv, "wp": wp,
                "mask": mask, "mask01": mask01,
            }
        )
    return in_maps


def kernel(x, Wq, Wk, Wv, Wp, bp, _R=1, _return_res=False):
    from concourse.bass_utils import run_bass_kernel_spmd

    nc = _get_program(_R)
    in_maps = _shard_inputs(x, Wq, Wk, Wv, Wp)
    res = run_bass_kernel_spmd(nc, in_maps, list(range(8)))
    out = np.empty((B, S, D), dtype=np.float32)
    for b in range(B):
        p0 = res.results[2 * b]["out"].reshape(S, D)
        p1 = res.results[2 * b + 1]["out"].reshape(S, D)
        out[b] = p0 + p1 + bp.astype(np.float32)
    if _return_res:
        return out, res
    return out
